# revision 27
# baseline (speedup 1.0000x reference)
"""CRF loss (nn_CRFLossOld) on 8 Trainium2 NeuronCores.

Forward/backward-split linear-domain CRF:

  - Sequences are sorted by length (desc) and dealt round-robin to the 8
    cores; per core, rank q -> (group g=q%2, slot c=q//2) so both
    partition-halves of each tile carry 64 columns.
  - FORWARD chain (H=258 steps): a_j = (E2f^T a_{j-1}) * exp(obs_j) with
    E2f a 128x128 block-diagonal bf16 stationary (two copies of the
    64x64 exp'd transition matrix; rows 0/1 of each block are the
    keep-alive plumbing, =1). Short sequences (s+1 <= H) complete here
    (extraction at s+1, then keep-alive holds w in rows 0/1).
  - BACKWARD chain (NJB=256 steps) for long sequences, run on the
    e-shifted suffix state bt_j = e_j . beta_j, time-reversed and
    delayed so ALL columns finish at step NJB with beta_H:
      r < r0=513-s: hold [1,1,0..]   r==r0: inject bt_{s+1}=[1,0,..]
      r0<r<NJB: live (global time 513-r)   r==NJB: all-ones e => beta_H
    Stationary M2b = block-diag exp(trans^T) with plumbing rows/cols
    (see _build_program).
  - MERGE: per column Z = sum_k alpha_H[k]*gamma[k]; gamma = beta_H for
    long slots (first W_b free slots), [1,1,0..] for the rest. The dot
    telescopes both chains' per-column rescale shifts.
  - A constant per-step bias e^-C (C~4.4, the mean growth rate) is
    folded into the obs exp; dead/hold plumbing rows are host-boosted
    by +C so they stay exactly stable. This bounds per-column drift to
    ~e^+-20, allowing rescales only every R=32 steps (staggered across
    the three chain objects to avoid event spikes) and a raw-state
    merge with a single 2^-32-prescaled Ln (no final normalization).
    Boosted-step counts ship as a per-core scalar and are corrected on
    device. Rescale events: per-group colsum via a [128,2] selector
    matmul, raw cs stashed with an ACT Copy (all event-path ACT ops
    stay in one activation table - no table reloads), 1/cs via DVE
    reciprocal, partition-broadcast via a [2,128] matmul, applied
    LAG=4 steps later by premultiplying the exp(obs) arena slice on
    GPSIMD (SBUF-only operands). All ln's run as two batched ACT ops
    at the endgame.
  - Chain arithmetic is bf16 (PE 1 cycle/row; matmuls accumulate fp32
    in PSUM), exact bookkeeping in fp32 shifts.
  - Gold path score: emissions are host-gathered (dtype-preserving
    integer-indexed copy of pred) and summed with one ACT accumulate;
    transitions via per-core integer pair-count matrix dotted with the
    permuted transition table on device.

Host-side prep is integer-derived only: dtype-preserving copies
(packing, gather, transpose), constant fills, and integer counts. All
float arithmetic on pred/transitions happens on-device.
"""

import os
import sys

for _p in ("/opt/trn_rl_repo", "/root/.axon_site/_ro/trn_rl_repo"):
    if os.path.isdir(_p) and _p not in sys.path:
        sys.path.insert(0, _p)

import numpy as np

B, T, L = 1024, 512, 62
K = 64
NCORES = 8
BC = 128                     # sequences per core
H = 258                      # forward steps
NJB = 513 - H + 1            # backward steps: inject + (s-H) live + final
SMALL = -1000.0
R_RESCALE = 32
LAG = 8
CBIAS = float(np.float32(4.4))   # per-step drift cancel: e tiles scaled e^-C
CH = 8                       # steps per DMA/exp chunk (small => ACT queue
                             # stays fine-grained; rescale ops don't stall)
LOOKAHEAD = 3                # chunks issued ahead of the chain
LN2_32 = 32.0 * float(np.log(2.0))

# permutation: new index k' -> old label index (0=end, 1=start, 2+l=label l)
PERM = np.concatenate(([63, 62], np.arange(62)))

_PROGRAM_CACHE = {}


# --------------------------------------------------------------------------
# host-side packing (integer-derived only)
# --------------------------------------------------------------------------

def _events(n_steps, offset=0):
    """Staggered rescale decision steps; apply (at +LAG) must land."""
    return list(range(R_RESCALE + offset, n_steps - LAG - 1, R_RESCALE))


def _pack(pred, ref, seq_len):
    pred = np.ascontiguousarray(pred, dtype=np.float32)
    ref64 = np.asarray(ref).astype(np.int64)
    s_all = np.asarray(seq_len).astype(np.int64)

    order = np.argsort(-s_all, kind="stable")
    assign = np.empty((NCORES, BC), dtype=np.int64)
    assign[np.arange(B) % NCORES, np.arange(B) // NCORES] = order
    s = s_all[assign]                                   # (C, 128)
    nlong = (s >= H).sum(axis=1)
    W_b = max(1, int(np.ceil(nlong.max() / 2)))

    # ---- forward obs [C, H, 128, 64] -------------------------------------
    obs_f = np.empty((NCORES, H, 128, 64), np.float32)
    obs_f.fill(SMALL)
    jj = np.arange(H)
    for g in (0, 1):
        qs = np.arange(g, BC, 2)
        idx = assign[:, qs]                             # (C, 64)
        sg = s[:, qs]                                   # (C, 64)
        vals = pred[idx, :H, :]                         # (C, 64, H, 62)
        live = jj[None, :, None] < sg[:, None, :]       # (C, H, 64)
        obs_f[:, :, 64 * g + 2 : 64 * g + 64, :] = np.where(
            live[:, :, None, :], vals.transpose(0, 2, 3, 1), np.float32(SMALL)
        )
        # extraction row (k'=0): jj == s  ->  obs 0.0
        ext = jj[None, :, None] == sg[:, None, :]
        obs_f[:, :, 64 * g + 0, :] = np.where(ext, 0.0, SMALL).astype(np.float32)
        # death rows (k'=1): jj > s -> keep-alive +C (cancels the e^-C bias)
        dead = jj[None, :, None] > sg[:, None, :]
        obs_f[:, :, 64 * g + 1, :] = np.where(dead, CBIAS, SMALL).astype(np.float32)

    # ---- backward obs [C, NJB, 128, W_b] ---------------------------------
    obs_b = np.empty((NCORES, NJB, 128, W_b), np.float32)
    obs_b.fill(SMALL)
    rr = np.arange(NJB)                                  # rr = r-1
    for g in (0, 1):
        qs = np.arange(g, 2 * W_b, 2)
        idx = assign[:, qs]                              # (C, W_b)
        sg = s[:, qs]                                    # (C, W_b)
        long = sg >= H                                   # (C, W_b)
        r0 = 513 - sg                                    # valid where long
        r = rr[None, :, None] + 1                        # (1, NJB, 1)
        # emission rows: live where long & r0 < r < NJB; time = 513-r
        liv = long[:, None, :] & (r > r0[:, None, :]) & (r < NJB)
        vals = pred[idx, ::-1, :][:, :, : NJB - 1, :]    # (C, W_b, NJB-1, 62)
        blk = np.where(
            liv[:, : NJB - 1, None, :],
            vals.transpose(0, 2, 3, 1),
            np.float32(SMALL),
        )
        obs_b[:, : NJB - 1, 64 * g + 2 : 64 * g + 64, :] = blk
        # row 0 (end'): +C on hold (r < r0) or short-hold, 0.0 at inject
        hold = np.where(long[:, None, :], r < r0[:, None, :], True)
        inj = long[:, None, :] & (r == r0[:, None, :])
        row0v = np.where(hold, CBIAS, np.where(inj, 0.0, SMALL))
        obs_b[:, :, 64 * g + 0, :] = row0v.astype(np.float32)
        # row 1 (keep-alive): +C on hold only
        obs_b[:, :, 64 * g + 1, :] = np.where(hold, CBIAS, SMALL).astype(np.float32)
    obs_b[:, NJB - 1, :, :] = 0.0                        # final all-ones step

    # ---- gold emissions (host integer gather, masked) --------------------
    pred_d = pred[assign]                                # (C, 128, T, 62)
    gold = np.take_along_axis(pred_d, ref64[assign][..., None], axis=3)[..., 0]
    tmask = np.arange(T)[None, None, :] < s[:, :, None]
    gold = np.where(tmask, gold, np.float32(0.0)).astype(np.float32)

    # ---- gold transition pair counts (permuted space) --------------------
    cmat = np.zeros((NCORES, K, K), dtype=np.int64)
    ref_d = ref64[assign]                                # (C, 128, T)
    for c in range(NCORES):
        for q in range(BC):
            sq = int(s[c, q])
            path = np.concatenate(([1], ref_d[c, q, :sq] + 2, [0]))
            np.add.at(cmat[c], (path[:-1], path[1:]), 1)

    # boosted-step counts per core (for the e^-C bias correction)
    nb_f = np.maximum(0, H - s - 1).sum(axis=1)                  # (C,)
    sq_b = s[:, : 2 * W_b]
    nb_b = np.where(sq_b >= H, 512 - sq_b, NJB - 1).sum(axis=1)  # (C,)
    bcount = (nb_f + nb_b).astype(np.float32).reshape(NCORES, 1, 1)
    return obs_f, obs_b, gold, cmat.astype(np.float32), W_b, bcount


# --------------------------------------------------------------------------
# device program
# --------------------------------------------------------------------------

class _Chain:
    """One serial chain object: state tile + pools + rescale bookkeeping."""

    def __init__(self, name, w, c0, lhsT, arena, lnbuf, lcol0, apool, ppool):
        self.name = name
        self.w = w                  # free width
        self.c0 = c0                # slot offset inside arena/lnbuf
        self.lhsT = lhsT
        self.arena = arena
        self.lnbuf = lnbuf
        self.lcol0 = lcol0          # column offset inside lnbuf
        self.apool = apool
        self.ppool = ppool
        self.a = None
        self.pending = {}           # apply_step -> bcs sbuf tile


def _build_program(W_b):
    import concourse.bacc as bacc
    import concourse.tile as tile
    from concourse import mybir

    f32 = mybir.dt.float32
    bf16 = mybir.dt.bfloat16
    AF = mybir.ActivationFunctionType
    ALU = mybir.AluOpType
    AX = mybir.AxisListType

    ev_f0 = _events(H, 0)
    ev_f1 = _events(H, 10)
    ev_b = _events(NJB, 21)
    assert len(ev_f0) == len(ev_f1)
    nev_f, nev_b = len(ev_f0), len(ev_b)

    nc = bacc.Bacc()
    obsf_d = nc.dram_tensor("obsf", [H, 128, 64], f32, kind="ExternalInput")
    obsb_d = nc.dram_tensor("obsb", [NJB, 128, W_b], f32, kind="ExternalInput")
    gold_d = nc.dram_tensor("gold", [128, T], f32, kind="ExternalInput")
    trans_d = nc.dram_tensor("trans", [K, K], f32, kind="ExternalInput")
    transfp_d = nc.dram_tensor("transfp", [K, K], f32, kind="ExternalInput")
    transb_d = nc.dram_tensor("transb", [K, K], f32, kind="ExternalInput")
    bcw_d = nc.dram_tensor("bcwp", [2, 128], f32, kind="ExternalInput")
    bcount_d = nc.dram_tensor("bcount", [1, 1], f32, kind="ExternalInput")
    cmat_d = nc.dram_tensor("cmat", [K, K], f32, kind="ExternalInput")
    out_d = nc.dram_tensor("out", [1, 8], f32, kind="ExternalOutput")

    with tile.TileContext(nc) as tc:
        with (
            tc.tile_pool(name="const", bufs=1) as const,
            tc.tile_pool(name="stgf", bufs=3) as stgf,
            tc.tile_pool(name="stgb", bufs=3) as stgb,
            tc.tile_pool(name="aF0", bufs=3) as aF0p,
            tc.tile_pool(name="aF1", bufs=3) as aF1p,
            tc.tile_pool(name="aB0", bufs=3) as aB0p,
            tc.tile_pool(name="rsc", bufs=4) as rsc,
            tc.tile_pool(name="endp", bufs=1) as endp,
            tc.tile_pool(name="pF0", bufs=1, space="PSUM") as pF0,
            tc.tile_pool(name="pF1", bufs=1, space="PSUM") as pF1,
            tc.tile_pool(name="pB0", bufs=1, space="PSUM") as pB0,
            tc.tile_pool(name="pmisc", bufs=1, space="PSUM") as pmisc,
        ):
            # ---- constants -----------------------------------------------
            trans_s = const.tile([K, K], f32)
            nc.gpsimd.dma_start(out=trans_s, in_=trans_d[:, :])
            transfp_s = const.tile([K, K], f32)
            nc.gpsimd.dma_start(out=transfp_s, in_=transfp_d[:, :])
            transb_s = const.tile([K, K], f32)
            nc.gpsimd.dma_start(out=transb_s, in_=transb_d[:, :])
            cmat_s = const.tile([K, K], f32)
            nc.gpsimd.dma_start(out=cmat_s, in_=cmat_d[:, :])
            bcw_f32 = const.tile([2, 128], f32)
            nc.gpsimd.dma_start(out=bcw_f32, in_=bcw_d[:, :])
            bcount_s = const.tile([1, 1], f32)
            nc.gpsimd.dma_start(out=bcount_s, in_=bcount_d[:, :])

            # stationaries: block-diag exp() of the host-plumbed matrices
            E2f = const.tile([128, 128], bf16)
            nc.vector.memset(E2f, 0.0)
            M2b = const.tile([128, 128], bf16)
            nc.vector.memset(M2b, 0.0)
            for g in (0, 1):
                o = 64 * g
                nc.scalar.activation(
                    out=E2f[o : o + K, o : o + K], in_=transfp_s, func=AF.Exp
                )
                nc.scalar.activation(
                    out=M2b[o : o + K, o : o + K], in_=transb_s, func=AF.Exp
                )

            # per-group colsum selector [128, 2] and broadcast weights [2, 128]
            sel2 = const.tile([128, 2], bf16)
            nc.vector.memset(sel2, 0.0)
            nc.vector.memset(sel2[0:64, 0:1], 1.0)
            nc.vector.memset(sel2[64:128, 1:2], 1.0)
            sel2f = const.tile([128, 2], f32)
            nc.vector.memset(sel2f, 0.0)
            nc.vector.memset(sel2f[0:64, 0:1], 1.0)
            nc.vector.memset(sel2f[64:128, 1:2], 1.0)
            bcw = const.tile([2, 128], bf16)
            nc.scalar.copy(out=bcw, in_=bcw_f32)
            ones_col = const.tile([128, 1], f32)
            nc.vector.memset(ones_col, 1.0)
            ebias = const.tile([128, 1], f32)       # exp bias: -C per step
            nc.vector.memset(ebias, -CBIAS)

            # chain initial states
            a0f = const.tile([128, 64], bf16)
            nc.vector.memset(a0f, 1.0)
            nc.vector.memset(a0f[0:2, :], 0.0)
            nc.vector.memset(a0f[64:66, :], 0.0)
            b0b = const.tile([128, W_b], bf16)
            nc.vector.memset(b0b, 0.0)
            nc.vector.memset(b0b[0:2, :], 1.0)
            nc.vector.memset(b0b[64:66, :], 1.0)

            # raw per-event colsum stash (ln'd in one batched op at endgame)
            csbuf_f = const.tile([2, nev_f, 64], f32)
            csbuf_b = const.tile([2, nev_b, W_b], f32)

            # ---- exp arenas (bf16), streamed in CH-step chunks -----------
            e_f = const.tile([128, H, 64], bf16)
            e_b = const.tile([128, NJB, W_b], bf16)

            def _bounds(n):
                cuts = [0, 2, 4, 8]
                while cuts[-1] < n:
                    cuts.append(min(n, cuts[-1] + CH))
                return list(zip(cuts[:-1], cuts[1:]))

            bounds_f = _bounds(H)
            bounds_b = _bounds(NJB)
            nchunk_f = len(bounds_f)
            nchunk_b = len(bounds_b)

            def issue_chunk_f(ci):
                j0, j1 = bounds_f[ci]
                cw = j1 - j0
                st = stgf.tile([128, CH, 64], f32, tag="stf")
                nc.sync.dma_start(
                    out=st[:, :cw, :],
                    in_=obsf_d[j0:j1].rearrange("j p b -> p j b"),
                )
                nc.scalar.activation(
                    out=e_f[:, j0:j1, :], in_=st[:, :cw, :], func=AF.Exp,
                    bias=ebias,
                )

            def issue_chunk_b(ci):
                j0, j1 = bounds_b[ci]
                cw = j1 - j0
                st = stgb.tile([128, CH, W_b], f32, tag="stb")
                nc.sync.dma_start(
                    out=st[:, :cw, :],
                    in_=obsb_d[j0:j1].rearrange("j p b -> p j b"),
                )
                nc.scalar.activation(
                    out=e_b[:, j0:j1, :], in_=st[:, :cw, :], func=AF.Exp,
                    bias=ebias,
                )

            # ---- chain objects -------------------------------------------
            F0 = _Chain("F0", 32, 0, E2f, e_f, csbuf_f, 0, aF0p, pF0)
            F1 = _Chain("F1", 32, 32, E2f, e_f, csbuf_f, 32, aF1p, pF1)
            B0 = _Chain("B0", W_b, 0, M2b, e_b, csbuf_b, 0, aB0p, pB0)
            F0.a, F1.a = a0f[:, 0:32], a0f[:, 32:64]
            B0.a = b0b
            ev_sets = {"F0": (set(ev_f0), ev_f0), "F1": (set(ev_f1), ev_f1),
                       "B0": (set(ev_b), ev_b)}

            def chain_step(obj, j, ev_set, ev_list):
                # pending rescale apply: premultiply the arena slice (GPSIMD,
                # SBUF-only) BEFORE this step's mul reads it
                bcs = obj.pending.pop(j, None)
                esl = obj.arena[:, j - 1, obj.c0 : obj.c0 + obj.w]
                if bcs is not None:
                    nc.gpsimd.tensor_mul(esl, esl, bcs)
                ps = obj.ppool.tile([128, obj.w], f32, tag=f"ps{obj.name}")
                nc.tensor.matmul(
                    ps, lhsT=obj.lhsT, rhs=obj.a, start=True, stop=True
                )
                anew = obj.apool.tile([128, obj.w], bf16, tag=f"a{obj.name}")
                nc.vector.tensor_mul(anew, ps, esl)
                obj.a = anew
                if j in ev_set:
                    ri = ev_list.index(j)
                    cs_full = pmisc.tile([2, 64], f32, tag="cs")
                    cs = cs_full[:, : obj.w]
                    nc.tensor.matmul(
                        cs, lhsT=sel2, rhs=anew, start=True, stop=True
                    )
                    csl = obj.lnbuf[:, ri, obj.lcol0 : obj.lcol0 + obj.w]
                    nc.scalar.activation(out=csl, in_=cs, func=AF.Copy)
                    bcx = rsc.tile([2, obj.w], bf16, tag="bcx")
                    with nc.allow_low_precision(reason="rescale factor; drift lands in tracked shifts"):
                        nc.vector.reciprocal(bcx, cs)
                    psb_full = pmisc.tile([128, 64], f32, tag="psb")
                    psb = psb_full[:, : obj.w]
                    nc.tensor.matmul(
                        psb, lhsT=bcw, rhs=bcx, start=True, stop=True
                    )
                    bcs2 = rsc.tile([128, obj.w], bf16, tag="bcs")
                    nc.scalar.activation(out=bcs2, in_=psb, func=AF.Copy)
                    obj.pending[j + LAG] = bcs2

            # gold DMA early (background; compute happens post-chain)
            gold_s = const.tile([128, T], f32)
            nc.sync.dma_start(out=gold_s, in_=gold_d[:, :])

            # prologue chunks
            nf = nb = 0
            for _ in range(LOOKAHEAD):
                if nf < nchunk_f:
                    issue_chunk_f(nf)
                    nf += 1
                if nb < nchunk_b:
                    issue_chunk_b(nb)
                    nb += 1

            # gamma pattern (cheap, engines idle here)
            gm = const.tile([128, 64], f32)
            nc.vector.memset(gm, 0.0)
            nc.vector.memset(gm[0:2, :], 1.0)
            nc.vector.memset(gm[64:66, :], 1.0)
            gacc = const.tile([128, 1], f32)
            trans_st = const.tile([K, K], f32)
            cmat_st = const.tile([K, K], f32)
            scr = const.tile([K, K], f32)
            gt = const.tile([K, 1], f32)

            def issue_gold():
                # mid-chain: ACT has slack and the gold DMA finished long ago
                nc.scalar.activation(
                    out=gold_s, in_=gold_s, func=AF.Copy, accum_out=gacc
                )
                nc.scalar.copy(out=trans_st, in_=trans_s)
                nc.scalar.copy(out=cmat_st, in_=cmat_s)
                nc.vector.tensor_mul(scr, trans_st, cmat_st)
                nc.vector.tensor_reduce(out=gt, in_=scr, axis=AX.X, op=ALU.add)

            for it in range(1, H + 1):
                while nf < nchunk_f and bounds_f[nf][0] < it + LOOKAHEAD * CH:
                    issue_chunk_f(nf)
                    nf += 1
                while nb < nchunk_b and bounds_b[nb][0] < it + LOOKAHEAD * CH:
                    issue_chunk_b(nb)
                    nb += 1
                chain_step(F0, it, *ev_sets["F0"])
                chain_step(F1, it, *ev_sets["F1"])
                if it <= NJB:
                    chain_step(B0, it, *ev_sets["B0"])
                if it == 40:
                    issue_gold()

            # ---- endgame --------------------------------------------------
            # batched shift lns first: ACT works while DVE does the merge
            lnbuf_f = endp.tile([2, nev_f, 64], f32)
            nc.scalar.activation(
                out=lnbuf_f, in_=csbuf_f, func=AF.Ln, scale=float(2.0 ** -32)
            )
            lnbuf_b = endp.tile([2, nev_b, W_b], f32)
            nc.scalar.activation(
                out=lnbuf_b, in_=csbuf_b, func=AF.Ln, scale=float(2.0 ** -32)
            )

            # merge on the RAW final states: the e^-C bias bounds drift to
            # ~e^+-20, so the dot fits fp32 and one 2^-32-prescaled Ln
            # replaces the three final normalizations entirely
            nc.vector.tensor_copy(out=gm[:, 0:W_b], in_=B0.a)

            m0 = endp.tile([128, 64], f32)
            nc.vector.tensor_mul(m0[:, 0:32], F0.a, gm[:, 0:32])
            nc.vector.tensor_mul(m0[:, 32:64], F1.a, gm[:, 32:64])
            psd = pmisc.tile([2, 64], f32, tag="cs")
            nc.tensor.matmul(psd, lhsT=sel2f, rhs=m0, start=True, stop=True)
            lnd = endp.tile([2, 64], f32)
            nc.scalar.activation(
                out=lnd, in_=psd, func=AF.Ln, scale=float(2.0 ** -32)
            )

            ssf = endp.tile([2, 64], f32)
            nc.vector.tensor_reduce(
                out=ssf, in_=lnbuf_f.rearrange("p a b -> p b a"),
                axis=AX.X, op=ALU.add,
            )
            ssb = endp.tile([2, W_b], f32)
            nc.vector.tensor_reduce(
                out=ssb, in_=lnbuf_b.rearrange("p a b -> p b a"),
                axis=AX.X, op=ALU.add,
            )
            tot = endp.tile([2, 64], f32)
            nc.vector.tensor_add(tot, lnd, ssf)
            nc.vector.tensor_add(tot[:, 0:W_b], tot[:, 0:W_b], ssb)
            red = endp.tile([2, 1], f32)
            nc.vector.tensor_reduce(out=red, in_=tot, axis=AX.X, op=ALU.add)
            ones2 = endp.tile([2, 1], f32)
            nc.vector.memset(ones2, 1.0)
            zps = pmisc.tile([1, 1], f32, tag="sc1")
            nc.tensor.matmul(zps, lhsT=red, rhs=ones2, start=True, stop=True)
            bias = (
                BC * (LN2_32 * (nev_f + 1) - 1000.0 + CBIAS * H)
                + 2.0 * W_b * (LN2_32 * nev_b + CBIAS * NJB)
            )
            szl = endp.tile([1, 1], f32)
            nc.scalar.activation(
                out=szl, in_=zps, func=AF.Copy, bias=float(bias), scale=1.0
            )
            bcc = endp.tile([1, 1], f32)
            nc.scalar.activation(
                out=bcc, in_=bcount_s, func=AF.Copy, bias=0.0, scale=float(CBIAS)
            )
            nc.vector.tensor_sub(szl, szl, bcc)

            ge_ps = pmisc.tile([1, 1], f32, tag="sc1")
            nc.tensor.matmul(
                ge_ps, lhsT=gacc, rhs=ones_col, start=True, stop=True
            )
            gesb = endp.tile([1, 1], f32)
            nc.vector.tensor_copy(out=gesb, in_=ge_ps)
            gt_ps = pmisc.tile([1, 1], f32, tag="sc1")
            nc.tensor.matmul(
                gt_ps, lhsT=gt, rhs=ones_col[0:K, :], start=True, stop=True
            )

            fin = endp.tile([1, 8], f32)
            nc.vector.memset(fin, 0.0)
            nc.vector.tensor_sub(fin[:, 0:1], szl, gesb)
            nc.vector.tensor_sub(fin[:, 0:1], fin[:, 0:1], gt_ps)
            nc.sync.dma_start(out=out_d[:, :], in_=fin)

    nc.compile()
    return nc


def _get_program(W_b):
    if W_b not in _PROGRAM_CACHE:
        _PROGRAM_CACHE[W_b] = _build_program(W_b)
    return _PROGRAM_CACHE[W_b]


# --------------------------------------------------------------------------
# entry point
# --------------------------------------------------------------------------

def kernel(pred, ref, seq_len, transitions):
    from concourse.bass_utils import run_bass_kernel_spmd

    obs_f, obs_b, gold, cmat, W_b, bcount = _pack(pred, ref, seq_len)
    trans_f = np.ascontiguousarray(
        np.asarray(transitions, dtype=np.float32)[np.ix_(PERM, PERM)]
    )
    # plumbing as constant fills (mirrors the reference's own -10000 fills):
    # fwd: rows 0/1 (from end/keep-alive) -> 0.0 so exp()=1 keep-alive rows
    transf_p = trans_f.copy()
    transf_p[0, :] = 0.0
    transf_p[1, :] = 0.0
    # bwd stationary (lhsT = E^T with hold/inject plumbing):
    #   row1 = e1 (hold), [0,0]=1, [0,1]=0; rows>=2 cols 0/1 already -10000
    transb_p = np.ascontiguousarray(trans_f.T)
    transb_p[1, :] = -10000.0
    transb_p[1, 1] = 0.0
    transb_p[0, 0] = 0.0
    transb_p[0, 1] = -10000.0
    # partition-broadcast stationary pattern [2, 128]
    bcw_np = np.zeros((2, 128), np.float32)
    bcw_np[0, 0:64] = 1.0
    bcw_np[1, 64:128] = 1.0

    nc = _get_program(W_b)
    in_maps = [
        {
            "obsf": np.ascontiguousarray(obs_f[c]),
            "obsb": np.ascontiguousarray(obs_b[c]),
            "gold": np.ascontiguousarray(gold[c]),
            "trans": trans_f,
            "transfp": transf_p,
            "transb": transb_p,
            "bcwp": bcw_np,
            "cmat": np.ascontiguousarray(cmat[c]),
            "bcount": np.ascontiguousarray(bcount[c]),
        }
        for c in range(NCORES)
    ]
    total = np.float64(np.nan)
    for _attempt in range(3):
        res = run_bass_kernel_spmd(
            nc, in_maps, list(range(NCORES)),
            trace=bool(os.environ.get("BASS_TRACE")),
        )
        if res.exec_time_ns is not None:
            print(f"HW exec time: {res.exec_time_ns} ns")
        total = np.float64(0.0)
        for c in range(NCORES):
            total += np.float64(res.results[c]["out"][0, 0])
        if np.isfinite(total):
            break
    return np.array(np.float32(total))


# revision 28
# speedup vs baseline: 1.0428x; 1.0428x over previous
"""CRF loss (nn_CRFLossOld) on 8 Trainium2 NeuronCores.

Forward/backward-split linear-domain CRF:

  - Sequences are sorted by length (desc) and dealt round-robin to the 8
    cores; per core, rank q -> (group g=q%2, slot c=q//2) so both
    partition-halves of each tile carry 64 columns.
  - FORWARD chain (H=258 steps): a_j = (E2f^T a_{j-1}) * exp(obs_j) with
    E2f a 128x128 block-diagonal bf16 stationary (two copies of the
    64x64 exp'd transition matrix; rows 0/1 of each block are the
    keep-alive plumbing, =1). Short sequences (s+1 <= H) complete here
    (extraction at s+1, then keep-alive holds w in rows 0/1).
  - BACKWARD chain (NJB=256 steps) for long sequences, run on the
    e-shifted suffix state bt_j = e_j . beta_j, time-reversed and
    delayed so ALL columns finish at step NJB with beta_H:
      r < r0=513-s: hold [1,1,0..]   r==r0: inject bt_{s+1}=[1,0,..]
      r0<r<NJB: live (global time 513-r)   r==NJB: all-ones e => beta_H
    Stationary M2b = block-diag exp(trans^T) with plumbing rows/cols
    (see _build_program).
  - MERGE: per column Z = sum_k alpha_H[k]*gamma[k]; gamma = beta_H for
    long slots (first W_b free slots), [1,1,0..] for the rest. The dot
    telescopes both chains' per-column rescale shifts.
  - A constant per-step bias e^-C (C~4.4, the mean growth rate) is
    folded into the obs exp; dead/hold plumbing rows are host-boosted
    by +C so they stay exactly stable. This bounds per-column drift to
    ~e^+-20, allowing rescales only every R=32 steps (staggered across
    the three chain objects to avoid event spikes) and a raw-state
    merge with a single 2^-32-prescaled Ln (no final normalization).
    Boosted-step counts ship as a per-core scalar and are corrected on
    device. Rescale events: per-group colsum via a [128,2] selector
    matmul, raw cs stashed with an ACT Copy (all event-path ACT ops
    stay in one activation table - no table reloads), 1/cs via DVE
    reciprocal, partition-broadcast via a [2,128] matmul, applied
    LAG=4 steps later by premultiplying the exp(obs) arena slice on
    GPSIMD (SBUF-only operands). All ln's run as two batched ACT ops
    at the endgame.
  - Chain arithmetic is bf16 (PE 1 cycle/row; matmuls accumulate fp32
    in PSUM), exact bookkeeping in fp32 shifts.
  - Gold path score: emissions are host-gathered (dtype-preserving
    integer-indexed copy of pred) and summed with one ACT accumulate;
    transitions via per-core integer pair-count matrix dotted with the
    permuted transition table on device.

Host-side prep is integer-derived only: dtype-preserving copies
(packing, gather, transpose), constant fills, and integer counts. All
float arithmetic on pred/transitions happens on-device.
"""

import os
import sys

for _p in ("/opt/trn_rl_repo", "/root/.axon_site/_ro/trn_rl_repo"):
    if os.path.isdir(_p) and _p not in sys.path:
        sys.path.insert(0, _p)

import numpy as np

B, T, L = 1024, 512, 62
K = 64
NCORES = 8
BC = 128                     # sequences per core
H = 258                      # forward steps
NJB = 513 - H + 1            # backward steps: inject + (s-H) live + final
SMALL = -1000.0
R_RESCALE = 32
LAG = 4
CBIAS = float(np.float32(4.4))   # per-step drift cancel: e tiles scaled e^-C
CH = 8                       # steps per DMA/exp chunk (small => ACT queue
                             # stays fine-grained; rescale ops don't stall)
LOOKAHEAD = 3                # chunks issued ahead of the chain
LN2_32 = 32.0 * float(np.log(2.0))

# permutation: new index k' -> old label index (0=end, 1=start, 2+l=label l)
PERM = np.concatenate(([63, 62], np.arange(62)))

_PROGRAM_CACHE = {}


# --------------------------------------------------------------------------
# host-side packing (integer-derived only)
# --------------------------------------------------------------------------

def _events(n_steps, offset=0):
    """Staggered rescale decision steps; apply (at +LAG) must land."""
    return list(range(R_RESCALE + offset, n_steps - LAG - 1, R_RESCALE))


def _pack(pred, ref, seq_len):
    pred = np.ascontiguousarray(pred, dtype=np.float32)
    ref64 = np.asarray(ref).astype(np.int64)
    s_all = np.asarray(seq_len).astype(np.int64)

    order = np.argsort(-s_all, kind="stable")
    assign = np.empty((NCORES, BC), dtype=np.int64)
    assign[np.arange(B) % NCORES, np.arange(B) // NCORES] = order
    s = s_all[assign]                                   # (C, 128)
    nlong = (s >= H).sum(axis=1)
    W_b = max(1, int(np.ceil(nlong.max() / 2)))

    # ---- forward obs [C, H, 128, 64] -------------------------------------
    obs_f = np.empty((NCORES, H, 128, 64), np.float32)
    obs_f.fill(SMALL)
    jj = np.arange(H)
    for g in (0, 1):
        qs = np.arange(g, BC, 2)
        idx = assign[:, qs]                             # (C, 64)
        sg = s[:, qs]                                   # (C, 64)
        vals = pred[idx, :H, :]                         # (C, 64, H, 62)
        live = jj[None, :, None] < sg[:, None, :]       # (C, H, 64)
        obs_f[:, :, 64 * g + 2 : 64 * g + 64, :] = np.where(
            live[:, :, None, :], vals.transpose(0, 2, 3, 1), np.float32(SMALL)
        )
        # extraction row (k'=0): jj == s  ->  obs 0.0
        ext = jj[None, :, None] == sg[:, None, :]
        obs_f[:, :, 64 * g + 0, :] = np.where(ext, 0.0, SMALL).astype(np.float32)
        # death rows (k'=1): jj > s -> keep-alive +C (cancels the e^-C bias)
        dead = jj[None, :, None] > sg[:, None, :]
        obs_f[:, :, 64 * g + 1, :] = np.where(dead, CBIAS, SMALL).astype(np.float32)

    # ---- backward obs [C, NJB, 128, W_b] ---------------------------------
    obs_b = np.empty((NCORES, NJB, 128, W_b), np.float32)
    obs_b.fill(SMALL)
    rr = np.arange(NJB)                                  # rr = r-1
    for g in (0, 1):
        qs = np.arange(g, 2 * W_b, 2)
        idx = assign[:, qs]                              # (C, W_b)
        sg = s[:, qs]                                    # (C, W_b)
        long = sg >= H                                   # (C, W_b)
        r0 = 513 - sg                                    # valid where long
        r = rr[None, :, None] + 1                        # (1, NJB, 1)
        # emission rows: live where long & r0 < r < NJB; time = 513-r
        liv = long[:, None, :] & (r > r0[:, None, :]) & (r < NJB)
        vals = pred[idx, ::-1, :][:, :, : NJB - 1, :]    # (C, W_b, NJB-1, 62)
        blk = np.where(
            liv[:, : NJB - 1, None, :],
            vals.transpose(0, 2, 3, 1),
            np.float32(SMALL),
        )
        obs_b[:, : NJB - 1, 64 * g + 2 : 64 * g + 64, :] = blk
        # row 0 (end'): +C on hold (r < r0) or short-hold, 0.0 at inject
        hold = np.where(long[:, None, :], r < r0[:, None, :], True)
        inj = long[:, None, :] & (r == r0[:, None, :])
        row0v = np.where(hold, CBIAS, np.where(inj, 0.0, SMALL))
        obs_b[:, :, 64 * g + 0, :] = row0v.astype(np.float32)
        # row 1 (keep-alive): +C on hold only
        obs_b[:, :, 64 * g + 1, :] = np.where(hold, CBIAS, SMALL).astype(np.float32)
    obs_b[:, NJB - 1, :, :] = 0.0                        # final all-ones step

    # ---- gold emissions (host integer gather, masked) --------------------
    pred_d = pred[assign]                                # (C, 128, T, 62)
    gold = np.take_along_axis(pred_d, ref64[assign][..., None], axis=3)[..., 0]
    tmask = np.arange(T)[None, None, :] < s[:, :, None]
    gold = np.where(tmask, gold, np.float32(0.0)).astype(np.float32)

    # ---- gold transition pair counts (permuted space) --------------------
    cmat = np.zeros((NCORES, K, K), dtype=np.int64)
    ref_d = ref64[assign]                                # (C, 128, T)
    for c in range(NCORES):
        for q in range(BC):
            sq = int(s[c, q])
            path = np.concatenate(([1], ref_d[c, q, :sq] + 2, [0]))
            np.add.at(cmat[c], (path[:-1], path[1:]), 1)

    # boosted-step counts per core (for the e^-C bias correction)
    nb_f = np.maximum(0, H - s - 1).sum(axis=1)                  # (C,)
    sq_b = s[:, : 2 * W_b]
    nb_b = np.where(sq_b >= H, 512 - sq_b, NJB - 1).sum(axis=1)  # (C,)
    bcount = (nb_f + nb_b).astype(np.float32).reshape(NCORES, 1, 1)
    return obs_f, obs_b, gold, cmat.astype(np.float32), W_b, bcount


# --------------------------------------------------------------------------
# device program
# --------------------------------------------------------------------------

class _Chain:
    """One serial chain object: state tile + pools + rescale bookkeeping."""

    def __init__(self, name, w, c0, lhsT, arena, lnbuf, lcol0, apool, ppool):
        self.name = name
        self.w = w                  # free width
        self.c0 = c0                # slot offset inside arena/lnbuf
        self.lhsT = lhsT
        self.arena = arena
        self.lnbuf = lnbuf
        self.lcol0 = lcol0          # column offset inside lnbuf
        self.apool = apool
        self.ppool = ppool
        self.a = None
        self.pending = {}           # apply_step -> bcs sbuf tile


def _build_program(W_b):
    import concourse.bacc as bacc
    import concourse.tile as tile
    from concourse import mybir

    f32 = mybir.dt.float32
    bf16 = mybir.dt.bfloat16
    AF = mybir.ActivationFunctionType
    ALU = mybir.AluOpType
    AX = mybir.AxisListType

    ev_f0 = _events(H, 0)
    ev_f1 = _events(H, 10)
    ev_b = _events(NJB, 21)
    assert len(ev_f0) == len(ev_f1)
    nev_f, nev_b = len(ev_f0), len(ev_b)

    nc = bacc.Bacc()
    obsf_d = nc.dram_tensor("obsf", [H, 128, 64], f32, kind="ExternalInput")
    obsb_d = nc.dram_tensor("obsb", [NJB, 128, W_b], f32, kind="ExternalInput")
    gold_d = nc.dram_tensor("gold", [128, T], f32, kind="ExternalInput")
    trans_d = nc.dram_tensor("trans", [K, K], f32, kind="ExternalInput")
    transfp_d = nc.dram_tensor("transfp", [K, K], f32, kind="ExternalInput")
    transb_d = nc.dram_tensor("transb", [K, K], f32, kind="ExternalInput")
    bcw_d = nc.dram_tensor("bcwp", [2, 128], f32, kind="ExternalInput")
    bcount_d = nc.dram_tensor("bcount", [1, 1], f32, kind="ExternalInput")
    cmat_d = nc.dram_tensor("cmat", [K, K], f32, kind="ExternalInput")
    out_d = nc.dram_tensor("out", [1, 8], f32, kind="ExternalOutput")

    with tile.TileContext(nc) as tc:
        with (
            tc.tile_pool(name="const", bufs=1) as const,
            tc.tile_pool(name="stgf", bufs=3) as stgf,
            tc.tile_pool(name="stgb", bufs=3) as stgb,
            tc.tile_pool(name="aF0", bufs=3) as aF0p,
            tc.tile_pool(name="aF1", bufs=3) as aF1p,
            tc.tile_pool(name="aB0", bufs=3) as aB0p,
            tc.tile_pool(name="rsc", bufs=4) as rsc,
            tc.tile_pool(name="endp", bufs=1) as endp,
            tc.tile_pool(name="pF0", bufs=1, space="PSUM") as pF0,
            tc.tile_pool(name="pF1", bufs=1, space="PSUM") as pF1,
            tc.tile_pool(name="pB0", bufs=1, space="PSUM") as pB0,
            tc.tile_pool(name="pmisc", bufs=1, space="PSUM") as pmisc,
        ):
            # ---- constants -----------------------------------------------
            trans_s = const.tile([K, K], f32)
            nc.gpsimd.dma_start(out=trans_s, in_=trans_d[:, :])
            transfp_s = const.tile([K, K], f32)
            nc.gpsimd.dma_start(out=transfp_s, in_=transfp_d[:, :])
            transb_s = const.tile([K, K], f32)
            nc.gpsimd.dma_start(out=transb_s, in_=transb_d[:, :])
            cmat_s = const.tile([K, K], f32)
            nc.gpsimd.dma_start(out=cmat_s, in_=cmat_d[:, :])
            bcw_f32 = const.tile([2, 128], f32)
            nc.gpsimd.dma_start(out=bcw_f32, in_=bcw_d[:, :])
            bcount_s = const.tile([1, 1], f32)
            nc.gpsimd.dma_start(out=bcount_s, in_=bcount_d[:, :])

            # stationaries: block-diag exp() of the host-plumbed matrices
            E2f = const.tile([128, 128], bf16)
            nc.vector.memset(E2f, 0.0)
            M2b = const.tile([128, 128], bf16)
            nc.vector.memset(M2b, 0.0)
            for g in (0, 1):
                o = 64 * g
                nc.scalar.activation(
                    out=E2f[o : o + K, o : o + K], in_=transfp_s, func=AF.Exp
                )
                nc.scalar.activation(
                    out=M2b[o : o + K, o : o + K], in_=transb_s, func=AF.Exp
                )

            # per-group colsum selector [128, 2] and broadcast weights [2, 128]
            sel2 = const.tile([128, 2], bf16)
            nc.vector.memset(sel2, 0.0)
            nc.vector.memset(sel2[0:64, 0:1], 1.0)
            nc.vector.memset(sel2[64:128, 1:2], 1.0)
            sel2f = const.tile([128, 2], f32)
            nc.vector.memset(sel2f, 0.0)
            nc.vector.memset(sel2f[0:64, 0:1], 1.0)
            nc.vector.memset(sel2f[64:128, 1:2], 1.0)
            bcw = const.tile([2, 128], bf16)
            nc.scalar.copy(out=bcw, in_=bcw_f32)
            ones_col = const.tile([128, 1], f32)
            nc.vector.memset(ones_col, 1.0)
            ebias = const.tile([128, 1], f32)       # exp bias: -C per step
            nc.vector.memset(ebias, -CBIAS)

            # chain initial states
            a0f = const.tile([128, 64], bf16)
            nc.vector.memset(a0f, 1.0)
            nc.vector.memset(a0f[0:2, :], 0.0)
            nc.vector.memset(a0f[64:66, :], 0.0)
            b0b = const.tile([128, W_b], bf16)
            nc.vector.memset(b0b, 0.0)
            nc.vector.memset(b0b[0:2, :], 1.0)
            nc.vector.memset(b0b[64:66, :], 1.0)

            # raw per-event colsum stash (ln'd in one batched op at endgame)
            csbuf_f = const.tile([2, nev_f, 64], f32)
            csbuf_b = const.tile([2, nev_b, W_b], f32)

            # ---- exp arenas (bf16), streamed in CH-step chunks -----------
            e_f = const.tile([128, H, 64], bf16)
            e_b = const.tile([128, NJB, W_b], bf16)

            def _bounds(n):
                cuts = [0, 2, 4, 8]
                while cuts[-1] < n:
                    cuts.append(min(n, cuts[-1] + CH))
                return list(zip(cuts[:-1], cuts[1:]))

            bounds_f = _bounds(H)
            bounds_b = _bounds(NJB)
            nchunk_f = len(bounds_f)
            nchunk_b = len(bounds_b)

            def issue_chunk_f(ci):
                j0, j1 = bounds_f[ci]
                cw = j1 - j0
                st = stgf.tile([128, CH, 64], f32, tag="stf")
                nc.sync.dma_start(
                    out=st[:, :cw, :],
                    in_=obsf_d[j0:j1].rearrange("j p b -> p j b"),
                )
                nc.scalar.activation(
                    out=e_f[:, j0:j1, :], in_=st[:, :cw, :], func=AF.Exp,
                    bias=ebias,
                )

            def issue_chunk_b(ci):
                j0, j1 = bounds_b[ci]
                cw = j1 - j0
                st = stgb.tile([128, CH, W_b], f32, tag="stb")
                nc.sync.dma_start(
                    out=st[:, :cw, :],
                    in_=obsb_d[j0:j1].rearrange("j p b -> p j b"),
                )
                nc.scalar.activation(
                    out=e_b[:, j0:j1, :], in_=st[:, :cw, :], func=AF.Exp,
                    bias=ebias,
                )

            # ---- chain objects -------------------------------------------
            F0 = _Chain("F0", 32, 0, E2f, e_f, csbuf_f, 0, aF0p, pF0)
            F1 = _Chain("F1", 32, 32, E2f, e_f, csbuf_f, 32, aF1p, pF1)
            B0 = _Chain("B0", W_b, 0, M2b, e_b, csbuf_b, 0, aB0p, pB0)
            F0.a, F1.a = a0f[:, 0:32], a0f[:, 32:64]
            B0.a = b0b
            ev_sets = {"F0": (set(ev_f0), ev_f0), "F1": (set(ev_f1), ev_f1),
                       "B0": (set(ev_b), ev_b)}

            def chain_step(obj, j, ev_set, ev_list):
                # pending rescale apply: premultiply the arena slice (GPSIMD,
                # SBUF-only) BEFORE this step's mul reads it
                bcs = obj.pending.pop(j, None)
                esl = obj.arena[:, j - 1, obj.c0 : obj.c0 + obj.w]
                if bcs is not None:
                    nc.gpsimd.tensor_mul(esl, esl, bcs)
                ps = obj.ppool.tile([128, obj.w], f32, tag=f"ps{obj.name}")
                nc.tensor.matmul(
                    ps, lhsT=obj.lhsT, rhs=obj.a, start=True, stop=True
                )
                anew = obj.apool.tile([128, obj.w], bf16, tag=f"a{obj.name}")
                nc.vector.tensor_mul(anew, ps, esl)
                obj.a = anew
                if j in ev_set:
                    ri = ev_list.index(j)
                    cs_full = pmisc.tile([2, 64], f32, tag="cs")
                    cs = cs_full[:, : obj.w]
                    nc.tensor.matmul(
                        cs, lhsT=sel2, rhs=anew, start=True, stop=True
                    )
                    csl = obj.lnbuf[:, ri, obj.lcol0 : obj.lcol0 + obj.w]
                    nc.scalar.activation(out=csl, in_=cs, func=AF.Copy)
                    bcx = rsc.tile([2, obj.w], bf16, tag="bcx")
                    with nc.allow_low_precision(reason="rescale factor; drift lands in tracked shifts"):
                        nc.vector.reciprocal(bcx, cs)
                    psb_full = pmisc.tile([128, 64], f32, tag="psb")
                    psb = psb_full[:, : obj.w]
                    nc.tensor.matmul(
                        psb, lhsT=bcw, rhs=bcx, start=True, stop=True
                    )
                    bcs2 = rsc.tile([128, obj.w], bf16, tag="bcs")
                    nc.scalar.activation(out=bcs2, in_=psb, func=AF.Copy)
                    obj.pending[j + LAG] = bcs2

            # gold DMA early (background; compute happens post-chain)
            gold_s = const.tile([128, T], f32)
            nc.sync.dma_start(out=gold_s, in_=gold_d[:, :])

            # prologue chunks
            nf = nb = 0
            for _ in range(LOOKAHEAD):
                if nf < nchunk_f:
                    issue_chunk_f(nf)
                    nf += 1
                if nb < nchunk_b:
                    issue_chunk_b(nb)
                    nb += 1

            # gamma pattern (cheap, engines idle here)
            gm = const.tile([128, 64], f32)
            nc.vector.memset(gm, 0.0)
            nc.vector.memset(gm[0:2, :], 1.0)
            nc.vector.memset(gm[64:66, :], 1.0)
            gacc = const.tile([128, 1], f32)
            trans_st = const.tile([K, K], f32)
            cmat_st = const.tile([K, K], f32)
            scr = const.tile([K, K], f32)
            gt = const.tile([K, 1], f32)

            def issue_gold():
                # mid-chain: ACT has slack and the gold DMA finished long ago
                nc.scalar.activation(
                    out=gold_s, in_=gold_s, func=AF.Copy, accum_out=gacc
                )
                nc.scalar.copy(out=trans_st, in_=trans_s)
                nc.scalar.copy(out=cmat_st, in_=cmat_s)
                nc.vector.tensor_mul(scr, trans_st, cmat_st)
                nc.vector.tensor_reduce(out=gt, in_=scr, axis=AX.X, op=ALU.add)

            for it in range(1, H + 1):
                while nf < nchunk_f and bounds_f[nf][0] < it + LOOKAHEAD * CH:
                    issue_chunk_f(nf)
                    nf += 1
                while nb < nchunk_b and bounds_b[nb][0] < it + LOOKAHEAD * CH:
                    issue_chunk_b(nb)
                    nb += 1
                chain_step(F0, it, *ev_sets["F0"])
                chain_step(F1, it, *ev_sets["F1"])
                if it <= NJB:
                    chain_step(B0, it, *ev_sets["B0"])
                if it == 40:
                    issue_gold()

            # ---- endgame --------------------------------------------------
            # batched shift lns first: ACT works while DVE does the merge
            lnbuf_f = endp.tile([2, nev_f, 64], f32)
            nc.scalar.activation(
                out=lnbuf_f, in_=csbuf_f, func=AF.Ln, scale=float(2.0 ** -32)
            )
            lnbuf_b = endp.tile([2, nev_b, W_b], f32)
            nc.scalar.activation(
                out=lnbuf_b, in_=csbuf_b, func=AF.Ln, scale=float(2.0 ** -32)
            )

            # merge on the RAW final states: the e^-C bias bounds drift to
            # ~e^+-20, so the dot fits fp32 and one 2^-32-prescaled Ln
            # replaces the three final normalizations entirely
            nc.vector.tensor_copy(out=gm[:, 0:W_b], in_=B0.a)

            m0 = endp.tile([128, 64], f32)
            nc.vector.tensor_mul(m0[:, 0:32], F0.a, gm[:, 0:32])
            nc.vector.tensor_mul(m0[:, 32:64], F1.a, gm[:, 32:64])
            psd = pmisc.tile([2, 64], f32, tag="cs")
            nc.tensor.matmul(psd, lhsT=sel2f, rhs=m0, start=True, stop=True)
            lnd = endp.tile([2, 64], f32)
            nc.scalar.activation(
                out=lnd, in_=psd, func=AF.Ln, scale=float(2.0 ** -32)
            )

            ssf = endp.tile([2, 64], f32)
            nc.vector.tensor_reduce(
                out=ssf, in_=lnbuf_f.rearrange("p a b -> p b a"),
                axis=AX.X, op=ALU.add,
            )
            ssb = endp.tile([2, W_b], f32)
            nc.vector.tensor_reduce(
                out=ssb, in_=lnbuf_b.rearrange("p a b -> p b a"),
                axis=AX.X, op=ALU.add,
            )
            tot = endp.tile([2, 64], f32)
            nc.vector.tensor_add(tot, lnd, ssf)
            nc.vector.tensor_add(tot[:, 0:W_b], tot[:, 0:W_b], ssb)
            red = endp.tile([2, 1], f32)
            nc.vector.tensor_reduce(out=red, in_=tot, axis=AX.X, op=ALU.add)
            ones2 = endp.tile([2, 1], f32)
            nc.vector.memset(ones2, 1.0)
            zps = pmisc.tile([1, 1], f32, tag="sc1")
            nc.tensor.matmul(zps, lhsT=red, rhs=ones2, start=True, stop=True)
            bias = (
                BC * (LN2_32 * (nev_f + 1) - 1000.0 + CBIAS * H)
                + 2.0 * W_b * (LN2_32 * nev_b + CBIAS * NJB)
            )
            szl = endp.tile([1, 1], f32)
            nc.scalar.activation(
                out=szl, in_=zps, func=AF.Copy, bias=float(bias), scale=1.0
            )
            bcc = endp.tile([1, 1], f32)
            nc.scalar.activation(
                out=bcc, in_=bcount_s, func=AF.Copy, bias=0.0, scale=float(CBIAS)
            )
            nc.vector.tensor_sub(szl, szl, bcc)

            ge_ps = pmisc.tile([1, 1], f32, tag="sc1")
            nc.tensor.matmul(
                ge_ps, lhsT=gacc, rhs=ones_col, start=True, stop=True
            )
            gesb = endp.tile([1, 1], f32)
            nc.vector.tensor_copy(out=gesb, in_=ge_ps)
            gt_ps = pmisc.tile([1, 1], f32, tag="sc1")
            nc.tensor.matmul(
                gt_ps, lhsT=gt, rhs=ones_col[0:K, :], start=True, stop=True
            )

            fin = endp.tile([1, 8], f32)
            nc.vector.memset(fin, 0.0)
            nc.vector.tensor_sub(fin[:, 0:1], szl, gesb)
            nc.vector.tensor_sub(fin[:, 0:1], fin[:, 0:1], gt_ps)
            nc.sync.dma_start(out=out_d[:, :], in_=fin)

    nc.compile()
    return nc


def _get_program(W_b):
    if W_b not in _PROGRAM_CACHE:
        _PROGRAM_CACHE[W_b] = _build_program(W_b)
    return _PROGRAM_CACHE[W_b]


# --------------------------------------------------------------------------
# entry point
# --------------------------------------------------------------------------

def kernel(pred, ref, seq_len, transitions):
    from concourse.bass_utils import run_bass_kernel_spmd

    obs_f, obs_b, gold, cmat, W_b, bcount = _pack(pred, ref, seq_len)
    trans_f = np.ascontiguousarray(
        np.asarray(transitions, dtype=np.float32)[np.ix_(PERM, PERM)]
    )
    # plumbing as constant fills (mirrors the reference's own -10000 fills):
    # fwd: rows 0/1 (from end/keep-alive) -> 0.0 so exp()=1 keep-alive rows
    transf_p = trans_f.copy()
    transf_p[0, :] = 0.0
    transf_p[1, :] = 0.0
    # bwd stationary (lhsT = E^T with hold/inject plumbing):
    #   row1 = e1 (hold), [0,0]=1, [0,1]=0; rows>=2 cols 0/1 already -10000
    transb_p = np.ascontiguousarray(trans_f.T)
    transb_p[1, :] = -10000.0
    transb_p[1, 1] = 0.0
    transb_p[0, 0] = 0.0
    transb_p[0, 1] = -10000.0
    # partition-broadcast stationary pattern [2, 128]
    bcw_np = np.zeros((2, 128), np.float32)
    bcw_np[0, 0:64] = 1.0
    bcw_np[1, 64:128] = 1.0

    nc = _get_program(W_b)
    in_maps = [
        {
            "obsf": np.ascontiguousarray(obs_f[c]),
            "obsb": np.ascontiguousarray(obs_b[c]),
            "gold": np.ascontiguousarray(gold[c]),
            "trans": trans_f,
            "transfp": transf_p,
            "transb": transb_p,
            "bcwp": bcw_np,
            "cmat": np.ascontiguousarray(cmat[c]),
            "bcount": np.ascontiguousarray(bcount[c]),
        }
        for c in range(NCORES)
    ]
    total = np.float64(np.nan)
    for _attempt in range(3):
        res = run_bass_kernel_spmd(
            nc, in_maps, list(range(NCORES)),
            trace=bool(os.environ.get("BASS_TRACE")),
        )
        if res.exec_time_ns is not None:
            print(f"HW exec time: {res.exec_time_ns} ns")
        total = np.float64(0.0)
        for c in range(NCORES):
            total += np.float64(res.results[c]["out"][0, 0])
        if np.isfinite(total):
            break
    return np.array(np.float32(total))
